# revision 8
# baseline (speedup 1.0000x reference)
"""Trainium2 Bass kernel for nn_CosineCentroidLoss (stable_ge2e loss).

Math (reference, with x:(C,S,D), wp=softplus(w)):
    sum_c  = x.sum(axis=1)                      (C,D)
    mean_c = sum_c/S ; mh = mean_c/||mean_c|| = sum_c/||sum_c||
    sim_neg[j,i,k] = <x[j,i], mh[k]> / ||x[j,i]||
    sim_pos[j,i]   = <x[j,i], loo>/(||x||*||loo||),  loo = (sum_c[j]-x[j,i])/(S-1)
                   = (dxs - rs) / (nx * sqrt(ns2 - 2*dxs + rs))
        with rs=||x||^2, dxs=<x,sum_c[j]>, ns2=||sum_c[j]||^2, nx=sqrt(rs)
    logits = wp*sim + b ; the diagonal (k==j) uses sim_pos.
    loss   = mean_ji [ logsumexp_k(logits[j,i,:]) - logits[j,i,j] ]
    The bias b cancels exactly, so it is dropped on device.

Sharding: classes (axis 0 of x) are split across 8 cores (128 each).
Each core computes its local centroids, all-gathers the normalized
transposed centroids in bf16, computes its (128*16, 1024) similarity
block with a bf16 PE matmul, and produces per-sample losses (128,16).
The host gathers and takes the mean.

Per-core layout trick: samples are loaded i-major — SBUF chunk i holds
x[:, i, :] so partition p == local class p for every chunk. This makes
all per-class scalars (nsc, ns2, Pdiag) plain per-partition values and
removes any core-id dependence from the program (pure SPMD).
"""

from contextlib import ExitStack

import numpy as np

import concourse.bass as bass
import concourse.mybir as mybir
import concourse.tile as tile
from concourse import bacc
from concourse.bass import ts
from concourse.bass_utils import run_bass_kernel_spmd
from concourse.masks import make_identity

f32 = mybir.dt.float32
bf16 = mybir.dt.bfloat16
AF = mybir.ActivationFunctionType
AX = mybir.AxisListType

C, S, D = 1024, 16, 512
NCORES = 8
CL = C // NCORES  # 128 local classes
P = 128
NQ = D // P  # 4 contraction chunks
NH = C // 512  # 2 psum halves of the G row


def build_kernel(ctx: "ExitStack", tc: "tile.TileContext", x_d, wb_d, loss_d):
    nc = tc.nc

    const = ctx.enter_context(tc.tile_pool(name="const", bufs=1))
    stats = ctx.enter_context(tc.tile_pool(name="stats", bufs=1))
    xf_pool = ctx.enter_context(tc.tile_pool(name="xf", bufs=3))
    xb_pool = ctx.enter_context(tc.tile_pool(name="xb", bufs=S))
    xbt_pool = ctx.enter_context(tc.tile_pool(name="xbt", bufs=S))
    mh_pool = ctx.enter_context(tc.tile_pool(name="mh", bufs=1))
    junk_pool = ctx.enter_context(tc.tile_pool(name="junk", bufs=2))
    prod_pool = ctx.enter_context(tc.tile_pool(name="prod", bufs=2))
    psc_pool = ctx.enter_context(tc.tile_pool(name="psc", bufs=1, space="PSUM"))
    pg_pool = ctx.enter_context(tc.tile_pool(name="pg", bufs=3, space="PSUM"))
    dram = ctx.enter_context(tc.tile_pool(name="dram", bufs=1, space="DRAM"))

    # ---- constants ----
    ident = const.tile([P, P], bf16)
    make_identity(nc, ident)

    # wp = softplus(w) = ln(1 + exp(w)); Softplus has no act table on TRN2,
    # and everything below sticks to the exp/ln/square/copy table.
    wb_sb = const.tile([P, 1], f32)
    nc.sync.dma_start(wb_sb[:], wb_d)
    ew = const.tile([P, 1], f32)
    nc.scalar.activation(ew[:], wb_sb[:], AF.Exp)
    ew1 = const.tile([P, 1], f32)
    nc.vector.tensor_scalar_add(ew1[:], ew[:], 1.0)
    wp = const.tile([P, 1], f32)
    nc.scalar.activation(wp[:], ew1[:], AF.Ln)
    negwp = const.tile([P, 1], f32)
    nc.vector.tensor_scalar_mul(negwp[:], wp[:], -1.0)

    # ---- stage A: load chunks i-major, cast to bf16, rs, centroid sum ----
    rs = stats.tile([P, S], f32)  # ||x||^2 per sample
    psum_sc = psc_pool.tile([P, D], f32)  # sum_c (class p, d)
    xb = []
    for i in range(S):
        xf = xf_pool.tile([P, D], f32, tag="xf", name=f"xf{i}")
        nc.sync.dma_start(xf[:], x_d[:, i, :])
        xb_i = xb_pool.tile([P, D], bf16, tag="xb", name=f"xb{i}")
        nc.vector.tensor_copy(xb_i[:], xf[:])
        xb.append(xb_i)
        sqj = junk_pool.tile([P, D], bf16, tag="sqj", name=f"sqj{i}")
        nc.scalar.activation(
            sqj[:], xb_i[:], AF.Square, accum_out=rs[:, i : i + 1]
        )
        nc.tensor.matmul(
            psum_sc[:], ident[:], xb_i[:], start=(i == 0), stop=(i == S - 1)
        )

    # ---- stage B: normalize centroids, transpose, all-gather ----
    ns2 = stats.tile([P, 1], f32)
    sqj2 = junk_pool.tile([P, D], bf16, tag="sqj")
    nc.scalar.activation(sqj2[:], psum_sc[:], AF.Square, accum_out=ns2[:])
    # sqrt/rsqrt via exp(+-0.5*ln(.)) to stay on the exp/ln act table
    lnns2 = stats.tile([P, 1], f32)
    nc.scalar.activation(lnns2[:], ns2[:], AF.Ln)
    nsc = stats.tile([P, 1], f32)
    nc.scalar.activation(nsc[:], lnns2[:], AF.Exp, scale=0.5)
    rnsc = stats.tile([P, 1], f32)
    nc.scalar.activation(rnsc[:], lnns2[:], AF.Exp, scale=-0.5)
    mh_local = mh_pool.tile([P, D], bf16)
    nc.scalar.mul(mh_local[:], psum_sc[:], rnsc[:])

    mh_lt = mh_pool.tile([P, NQ, P], bf16)
    for q in range(NQ):
        nc.sync.dma_start(mh_lt[:, q, :], mh_local[:, ts(q, P)], transpose=True)

    cc_in = dram.tile([P, NQ * P], bf16)
    nc.sync.dma_start(cc_in[:], mh_lt[:])
    cc_out = dram.tile([NCORES * P, NQ * P], bf16, addr_space="Shared")
    nc.gpsimd.collective_compute(
        "AllGather",
        mybir.AluOpType.bypass,
        replica_groups=[list(range(NCORES))],
        ins=[cc_in[:]],
        outs=[cc_out[:]],
    )

    # ---- stage B' (overlaps collective): transposes, Pdiag, row scales ----
    xbt = []
    for i in range(S):
        xbt_i = xbt_pool.tile([P, NQ, P], bf16, tag="xbt", name=f"xbt{i}")
        for q in range(NQ):
            nc.sync.dma_start(xbt_i[:, q, :], xb[i][:, ts(q, P)], transpose=True)
        xbt.append(xbt_i)

    pd = stats.tile([P, S], f32)  # Pdiag = <x, mh_local> per sample
    for i in range(S):
        prod = prod_pool.tile([P, D], f32, tag="prod", name=f"prod{i}")
        nc.vector.tensor_mul(out=prod[:], in0=xb[i][:], in1=mh_local[:])
        nc.vector.reduce_sum(pd[:, i : i + 1], prod[:], axis=AX.X)

    lnrs = stats.tile([P, S], f32)
    nc.scalar.activation(lnrs[:], rs[:], AF.Ln)
    rnx = stats.tile([P, S], f32)
    nc.scalar.activation(rnx[:], lnrs[:], AF.Exp, scale=-0.5)  # 1/||x||
    s_t = stats.tile([P, S], f32)
    nc.vector.tensor_scalar_mul(s_t[:], rnx[:], wp[:])  # wp/nx

    # ---- stage C: scatter gathered centroids into mhT tiles ----
    mht = []
    for q in range(NQ):
        mht_q = mh_pool.tile([P, C], bf16, name=f"mht{q}")
        for c in range(NCORES):
            nc.sync.dma_start(
                mht_q[:, ts(c, P)], cc_out[ts(c, P), ts(q, P)]
            )
        mht.append(mht_q)

    # ---- stage D: main matmul G = x @ mh^T and fused exp+rowsum ----
    se = stats.tile([P, S], f32)  # sum_k exp(wp*sim - wp)
    for i in range(S):
        pg = pg_pool.tile([P, C], f32, tag="pg", name=f"pg{i}")
        for q in range(NQ):
            for h in range(NH):
                nc.tensor.matmul(
                    pg[:, ts(h, 512)],
                    xbt[i][:, q, :],
                    mht[q][:, ts(h, 512)],
                    start=(q == 0),
                    stop=(q == NQ - 1),
                )
        ej = junk_pool.tile([P, C], bf16, tag="ej", name=f"ej{i}")
        nc.scalar.activation(
            ej[:],
            pg[:],
            AF.Exp,
            bias=negwp[:],
            scale=s_t[:, i : i + 1],
            accum_out=se[:, i : i + 1],
        )

    # ---- stage E: batched per-sample tail on (P, S) tiles ----
    dxs = stats.tile([P, S], f32)
    nc.vector.tensor_scalar_mul(dxs[:], pd[:], nsc[:])  # <x, sum_c[j]>
    num = stats.tile([P, S], f32)
    nc.vector.tensor_sub(out=num[:], in0=dxs[:], in1=rs[:])
    t1 = stats.tile([P, S], f32)
    nc.vector.tensor_scalar_mul(t1[:], dxs[:], -2.0)
    t2 = stats.tile([P, S], f32)
    nc.vector.tensor_add(out=t2[:], in0=t1[:], in1=rs[:])
    den2 = stats.tile([P, S], f32)
    nc.vector.tensor_scalar_add(den2[:], t2[:], ns2[:])  # ||sum_c - x||^2
    # sim_pos = num / sqrt(rs * den2)
    q = stats.tile([P, S], f32)
    nc.vector.tensor_mul(out=q[:], in0=den2[:], in1=rs[:])
    lnq = stats.tile([P, S], f32)
    nc.scalar.activation(lnq[:], q[:], AF.Ln)
    rsq = stats.tile([P, S], f32)
    nc.scalar.activation(rsq[:], lnq[:], AF.Exp, scale=-0.5)
    simpos = stats.tile([P, S], f32)
    nc.vector.tensor_mul(out=simpos[:], in0=num[:], in1=rsq[:])

    posexp = stats.tile([P, S], f32)
    nc.scalar.activation(
        posexp[:], simpos[:], AF.Exp, bias=negwp[:], scale=wp[:]
    )  # exp(wp*sim_pos - wp)
    varg = stats.tile([P, S], f32)
    nc.vector.tensor_mul(out=varg[:], in0=s_t[:], in1=pd[:])  # diag neg logit
    correxp = stats.tile([P, S], f32)
    nc.scalar.activation(correxp[:], varg[:], AF.Exp, bias=negwp[:])

    se2 = stats.tile([P, S], f32)
    nc.vector.tensor_sub(out=se2[:], in0=se[:], in1=correxp[:])
    se3 = stats.tile([P, S], f32)
    nc.vector.tensor_add(out=se3[:], in0=se2[:], in1=posexp[:])
    lnse = stats.tile([P, S], f32)
    nc.scalar.activation(lnse[:], se3[:], AF.Ln)

    posv = stats.tile([P, S], f32)
    nc.vector.tensor_scalar_mul(posv[:], simpos[:], wp[:])
    l1 = stats.tile([P, S], f32)
    nc.vector.tensor_sub(out=l1[:], in0=lnse[:], in1=posv[:])
    loss_sb = stats.tile([P, S], f32)
    nc.vector.tensor_scalar_add(loss_sb[:], l1[:], wp[:])

    nc.sync.dma_start(loss_d, loss_sb[:])


def build_program():
    nc = bacc.Bacc(
        "TRN2",
        target_bir_lowering=False,
        debug=False,
        num_devices=NCORES,
    )
    x_d = nc.dram_tensor("x", [CL, S, D], f32, kind="ExternalInput").ap()
    wb_d = nc.dram_tensor("wb", [P, 1], f32, kind="ExternalInput").ap()
    loss_d = nc.dram_tensor("loss_part", [P, S], f32, kind="ExternalOutput").ap()

    with tile.TileContext(nc) as tc:
        with ExitStack() as ctx:
            build_kernel(ctx, tc, x_d, wb_d, loss_d)
    nc.compile()
    return nc


_PROGRAM = None


def _get_program():
    global _PROGRAM
    if _PROGRAM is None:
        _PROGRAM = build_program()
    return _PROGRAM


def make_in_maps(x: np.ndarray, w: np.ndarray):
    wb = np.full((P, 1), float(w.reshape(-1)[0]), dtype=np.float32)
    in_maps = []
    for c in range(NCORES):
        x_local = np.ascontiguousarray(x[c * CL : (c + 1) * CL], dtype=np.float32)
        in_maps.append({"x": x_local, "wb": wb})
    return in_maps


def kernel(x: np.ndarray, w: np.ndarray, b: np.ndarray) -> np.ndarray:
    # b shifts every logit equally and cancels in log_softmax - diag; unused.
    x = np.asarray(x, dtype=np.float32)
    nc = _get_program()
    in_maps = make_in_maps(x, np.asarray(w, dtype=np.float32))
    res = run_bass_kernel_spmd(nc, in_maps, list(range(NCORES)))
    parts = [res.results[c]["loss_part"] for c in range(NCORES)]
    total = np.concatenate([p.reshape(-1) for p in parts]).astype(np.float64)
    return np.float32(total.mean())
